# revision 52
# baseline (speedup 1.0000x reference)
"""Trainium2 Bass kernel for nn_CrossmodalFusion (B=1024, R=36, D=1024).

Data-parallel over the batch dim across 8 NeuronCores, with token-level
sparsity: the sigmoid attention mask zeroes every region token with
j >= region_lens[b], and for those tokens the output is exactly
relu(f1_b) independent of the data. The host therefore compacts each
core's valid tokens (in (batch, region) order), the device processes only
those (~51% of tokens), and the host scatters results back, filling
masked rows with a device-computed relu(f1_b) vector.

On-device activations are feature-major (features on SBUF partitions,
tokens on the free dim) so every matmul uses the small replicated weights
as the stationary lhsT operand, with no on-chip transposes of the big
activations. All FLOPs run on device.

Per-token attention scalars are reduced AND broadcast to all 128
partitions in one PE matmul (ones[nb,128].T @ masked-logits), so the
sigmoid weight row never leaves the chip. seg_rep ( = q_b for valid
tokens) is folded into the last matmul by K-augmentation:
out += qw_local.T @ ind, where qw = q @ f1_W is computed once per core.

Emission is software-pipelined: A(t) = {x DMA, MLP, attn logits} and
B(t) = {attn weights, sc, z, f1, store} are interleaved
A0, Q, A1, B0, A2, B1, ... so the PE always has independent work while
the short vector/scalar attention chain of the previous tile resolves.
The q-stage (Q) overlaps tile 0's MLP.

Because each core's token plan differs, 8 per-core programs are compiled
(concurrently) and dispatched asynchronously, one per NeuronCore.
"""
import os
import sys
import types
from concurrent.futures import ThreadPoolExecutor
from contextlib import ExitStack

sys.path.insert(0, "/opt/trn_rl_repo")

import numpy as np
import ml_dtypes

import concourse.bass as bass
import concourse.tile as tile
from concourse import bacc, mybir
from concourse.masks import make_identity

F32 = mybir.dt.float32
BF16 = mybir.dt.bfloat16
FP8 = mybir.dt.float8e4
DR = mybir.MatmulPerfMode.DoubleRow

NPBF16 = ml_dtypes.bfloat16
NPFP8 = mybir.dt.np(FP8)
FP8_SCALE = 4096.0

B, R, D = 1024, 36, 1024
H = D // 2
SEG_C = 133
NCORES = 8
BC = B // NCORES            # batches per core
KC = D // 128               # 8 feature chunks
KH = H // 128               # 4 hidden chunks

TOKCAP = 512                # tokens per tile (PSUM fp32 bank width)

LAST_EXEC_NS = None
_LAST_TRACE = None


def _wire_ntff_hook():
    if "antenv.axon_hooks" in sys.modules:
        return
    try:
        import trn_agent_boot.trn_boot as tb
        hook = tb._ntff_profile_via_ctypes("/opt/axon/libaxon_pjrt.so")
    except Exception:
        hook = None
    mod = types.ModuleType("antenv.axon_hooks")
    _h = [hook]
    mod.set_axon_ntff_profile_hook = lambda h: _h.__setitem__(0, h)
    mod.get_axon_ntff_profile_hook = lambda: _h[0]
    sys.modules["antenv.axon_hooks"] = mod


def _make_plan(lens_c):
    """Tile plan for one core from its per-batch valid-token counts.

    Returns (tiles, ntokc): tiles = list of dicts with t0, nt, b0, nb,
    segs = [(row_in_tile, lo, hi)] giving each local batch's token span
    inside the tile. Tiles are capped at TOKCAP tokens / 128 batches; a
    batch's tokens may split across adjacent tiles.
    """
    stream = []  # (local_batch, j)
    for lb, ln in enumerate(lens_c):
        stream.extend((lb, j) for j in range(int(ln)))
    ntokc = len(stream)
    tiles = []
    t0 = 0
    while t0 < ntokc:
        nt = 0
        b_first = stream[t0][0]
        while t0 + nt < ntokc and nt < TOKCAP:
            lb = stream[t0 + nt][0]
            if lb - b_first + 1 > 128:
                break
            nt += 1
        b_last = stream[t0 + nt - 1][0]
        segs = []
        pos = 0
        while pos < nt:
            lb = stream[t0 + pos][0]
            end = pos
            while end < nt and stream[t0 + end][0] == lb:
                end += 1
            segs.append((lb - b_first, pos, end))
            pos = end
        tiles.append(dict(t0=t0, nt=nt, b0=b_first, nb=b_last - b_first + 1, segs=segs))
        t0 += nt
    return tiles, ntokc


def _emit(ctx, tc, plan):
    nc = tc.nc
    AF = mybir.ActivationFunctionType
    ALU = mybir.AluOpType
    tiles, ntokc = plan
    T = len(tiles)

    # ---- DRAM I/O -------------------------------------------------------
    xT = nc.dram_tensor("xT", [D, ntokc], BF16, kind="ExternalInput").ap()
    unet = nc.dram_tensor("unet", [BC, SEG_C, 49], BF16, kind="ExternalInput").ap()
    ind_sz = sum(t["nb"] * t["nt"] for t in tiles)
    ind_blob = nc.dram_tensor("ind", [ind_sz], BF16, kind="ExternalInput").ap()
    wi = {}
    for name, shape, dt in [
        ("mi_W1", [D, H], BF16), ("mi_b1", [1, H], F32),
        ("mi_W2", [H, D], BF16), ("mi_b2", [1, D], F32),
        ("ms_W1", [D, H], BF16), ("ms_b1", [1, H], F32),
        ("ms_W2", [H, D], BF16), ("ms_b2", [1, D], F32),
        ("seg_W", [SEG_C, D], BF16), ("seg_b", [1, D], F32),
        ("ln_g", [1, D], BF16), ("ln_b", [1, D], BF16),
        ("sc_W8", [D, D], FP8), ("sc_b", [1, D], F32),
        ("f1_W", [D, D], BF16), ("f1_W8", [D, D], FP8), ("f1_b", [1, D], F32),
    ]:
        wi[name] = nc.dram_tensor(name, shape, dt, kind="ExternalInput").ap()
    outT = nc.dram_tensor("outT", [D, ntokc], BF16, kind="ExternalOutput").ap()
    fillv = nc.dram_tensor("fillv", [1, D], F32, kind="ExternalOutput").ap()

    # ---- persistent constants ------------------------------------------
    const = ctx.enter_context(tc.tile_pool(name="const", bufs=1))

    def load_w(name, kchunks, m, eng, dt=BF16):
        t = const.tile([128, kchunks, m], dt, tag=f"cw_{name}")
        eng.dma_start(t[:], wi[name].rearrange("(kc p) m -> p kc m", p=128))
        return t

    # scalar queue gets ONLY the two weights needed before its first
    # ACTIVATE (DMA issue blocks the issuing engine for the transfer);
    # the fp8 sc/f1 weights are loaded later, inside Qa, so they don't
    # compete with x0/unet for HBM bandwidth at startup.
    W_mi1 = load_w("mi_W1", KC, H, nc.scalar)
    W_mi2 = load_w("mi_W2", KH, D, nc.scalar)
    W_sc8 = None
    W_f18 = None

    def load_col(name, mchunks):
        # (1, mchunks*128) vector -> (128, mchunks) per-partition columns
        ap_ = wi[name]
        t = const.tile([128, mchunks], F32, tag=f"cc_{name}")
        src = bass.AP(tensor=ap_.tensor, offset=ap_.offset, ap=[[1, 128], [128, mchunks]])
        nc.gpsimd.dma_start(t[:], src)
        return t

    b_mi1c = load_col("mi_b1", KH)
    b_mi2c = load_col("mi_b2", KC)
    b_scc = load_col("sc_b", KC)
    b_f1c = load_col("f1_b", KC)

    ones_row = const.tile([1, 512], BF16)
    nc.vector.memset(ones_row[:], 1.0)
    ones_mat = const.tile([128, 128], BF16)
    nc.vector.memset(ones_mat[:], 1.0)
    ident_bf = const.tile([128, 128], BF16)
    make_identity(nc, ident_bf)
    eps_t = const.tile([128, 1], F32)
    nc.vector.memset(eps_t[:], 1e-5)

    qT_bf = const.tile([128, KC, BC], BF16)    # feature-major q (lhsT for attn)
    qw_bf = const.tile([BC, D], BF16)          # token-major q @ f1_W

    psum = ctx.enter_context(tc.tile_pool(name="psum", bufs=1, space="PSUM"))

    # ---- main-loop pools (entered before Q so A(0) can precede it) -----
    xp = ctx.enter_context(tc.tile_pool(name="xp", bufs=3))
    hp = ctx.enter_context(tc.tile_pool(name="hp", bufs=1))
    tp = ctx.enter_context(tc.tile_pool(name="tp", bufs=2))
    rp = ctx.enter_context(tc.tile_pool(name="rp", bufs=3))
    wcp = ctx.enter_context(tc.tile_pool(name="wcp", bufs=2))
    scp = ctx.enter_context(tc.tile_pool(name="scp", bufs=3))
    zp = ctx.enter_context(tc.tile_pool(name="zp", bufs=2))
    op = ctx.enter_context(tc.tile_pool(name="op", bufs=1))
    ip = ctx.enter_context(tc.tile_pool(name="ip", bufs=3))
    qlp = ctx.enter_context(tc.tile_pool(name="qlp", bufs=2))
    mp = ctx.enter_context(tc.tile_pool(name="mp", bufs=2))
    wbp = ctx.enter_context(tc.tile_pool(name="wbp", bufs=2))

    xT_r = xT.rearrange("(kc p) t -> p kc t", p=128)
    outT_r = outT.rearrange("(kc p) t -> p kc t", p=128)

    ind_offs = []
    off = 0
    for t in tiles:
        ind_offs.append(off)
        off += t["nb"] * t["nt"]

    x_tiles = [None] * T
    r_tiles = [None] * T
    ind_tiles = [None] * T
    at_tiles = [None] * T
    qw_tiles = [None] * T

    def emit_at(ti):
        tl = tiles[ti]
        nb, b0 = tl["nb"], tl["b0"]
        nt = tl["nt"]
        at = psum.tile([nb, nt], F32, tag="atps", bufs=2)
        for kc in range(KC):
            nc.tensor.matmul(at[:], qT_bf[:, kc, b0:b0 + nb], r_tiles[ti][:, kc, :],
                             start=(kc == 0), stop=(kc == KC - 1))
        at_tiles[ti] = at

    def emit_x(ti):
        tl = tiles[ti]
        t0, nt = tl["t0"], tl["nt"]
        x_bf = xp.tile([128, KC, nt], BF16, tag="x")
        # two halves so mm1's first k-chunks start on the first half
        nc.sync.dma_start(x_bf[:, 0:KC // 2, :], xT_r[:, 0:KC // 2, t0:t0 + nt])
        nc.sync.dma_start(x_bf[:, KC // 2:KC, :], xT_r[:, KC // 2:KC, t0:t0 + nt])
        x_tiles[ti] = x_bf

    def emit_A(ti, with_at=True):
        tl = tiles[ti]
        t0, nt, b0, nb = tl["t0"], tl["nt"], tl["b0"], tl["nb"]

        if x_tiles[ti] is None:
            emit_x(ti)
        x_bf = x_tiles[ti]

        ind = ip.tile([nb, nt], BF16, tag="ind")
        nc.gpsimd.dma_start(ind[:], bass.AP(tensor=ind_blob.tensor,
                                            offset=ind_blob.offset + ind_offs[ti],
                                            ap=[[nt, nb], [1, nt]]))
        ind_tiles[ti] = ind

        # mm1: h1 = relu(mi_W1.T @ x + b1)
        h1_bf = hp.tile([128, KH, nt], BF16, tag="h1")
        for mc in range(KH):
            sl = slice(mc * 128, (mc + 1) * 128)
            ps = psum.tile([128, nt], F32, tag="mmps", bufs=4)
            for kc in range(KC):
                nc.tensor.matmul(ps[:], W_mi1[:, kc, sl], x_bf[:, kc, :],
                                 start=(kc == 0), stop=(kc == KC - 1))
            nc.scalar.activation(h1_bf[:, mc, :], ps[:], AF.Relu, bias=b_mi1c[:, mc:mc + 1])

        # mm2: r = (mi_W2.T @ h1 + b2) + x
        r_bf = rp.tile([128, KC, nt], BF16, tag="r")
        for mc in range(KC):
            sl = slice(mc * 128, (mc + 1) * 128)
            ps = psum.tile([128, nt], F32, tag="mmps", bufs=4)
            for kc in range(KH):
                nc.tensor.matmul(ps[:], W_mi2[:, kc, sl], h1_bf[:, kc, :],
                                 start=(kc == 0), stop=(kc == KH - 1))
            tmp = tp.tile([128, nt], BF16, tag="mm2tmp")
            nc.scalar.activation(tmp[:], ps[:], AF.Identity, bias=b_mi2c[:, mc:mc + 1])
            nc.vector.tensor_add(r_bf[:, mc, :], tmp[:], x_bf[:, mc, :])
        r_tiles[ti] = r_bf

        if with_at:
            emit_at(ti)

    def emit_B(ti, with_at=False):
        tl = tiles[ti]
        t0, nt, b0, nb = tl["t0"], tl["nt"], tl["b0"], tl["nb"]
        if with_at:
            emit_at(ti)
        at = at_tiles[ti]
        ind = ind_tiles[ti]
        r_bf = r_tiles[ti]

        # per-tile local slice of qw (stationary base partition must be 0);
        # SBUF->SBUF DMA, consumed ~25us later by the f1 ind-matmul
        qw_loc = qlp.tile([nb, D], BF16, tag="qwloc")
        nc.gpsimd.dma_start(qw_loc[:], qw_bf[b0:b0 + nb, :])
        qw_tiles[ti] = qw_loc

        # attention: mask logits, reduce over block rows AND broadcast to
        # all 128 partitions in one matmul, sigmoid on-chip.
        masked = mp.tile([nb, nt], BF16, tag="msk")
        nc.vector.tensor_tensor(masked[:], at[:], ind[:], op=ALU.mult)
        bc = psum.tile([128, nt], F32, tag="bcps", bufs=1)
        nc.tensor.matmul(bc[:], ones_mat[0:nb, :], masked[:], start=True, stop=True)
        w_bc = wbp.tile([128, nt], BF16, tag="wbc")
        nc.scalar.activation(w_bc[:], bc[:], AF.Sigmoid, scale=float(1.0 / np.sqrt(D)))

        # wc = w * r (straight to fp8; feeds both the DoubleRow sc-matmul
        # and the z elementwise product). Tiles padded to TOKCAP so the
        # DoubleRow pair-stride stays 16B-aligned on partial tiles.
        wc_f8 = wcp.tile([128, KC, TOKCAP], FP8, tag="wc")
        for kc in range(KC):
            nc.vector.tensor_mul(wc_f8[:, kc, 0:nt], r_bf[:, kc, :], w_bc[:])
        # mm3 (fp8 DoubleRow, weights pre-scaled x4096):
        #   scaling = tanh(sc_W.T @ wc + sc_b); z = wc*scaling
        z_f8 = zp.tile([128, KC, TOKCAP], FP8, tag="z")
        for mc in range(KC):
            sl = slice(mc * 128, (mc + 1) * 128)
            ps = psum.tile([128, nt], F32, tag="mmps", bufs=4)
            for p2 in range(KC // 2):
                nc.tensor.matmul(ps[:], qstate["W_sc8"][:, 2 * p2:2 * p2 + 2, sl],
                                 wc_f8[:, 2 * p2:2 * p2 + 2, 0:nt],
                                 start=(p2 == 0), stop=(p2 == KC // 2 - 1),
                                 perf_mode=DR)
            sc_t = scp.tile([128, nt], BF16, tag="sc")
            nc.scalar.activation(sc_t[:], ps[:], AF.Tanh, bias=b_scc[:, mc:mc + 1],
                                 scale=float(1.0 / FP8_SCALE))
            nc.vector.tensor_mul(z_f8[:, mc, 0:nt], wc_f8[:, mc, 0:nt], sc_t[:])

        # mm4 (fp8 DoubleRow + bf16 ind-matmul, qw pre-scaled x4096):
        #   out = relu((f1_W8.T @ z + qw_local.T @ ind)/4096 + f1_b)
        o_bf = op.tile([128, KC, nt], BF16, tag="o")
        store_split = ti >= T - 2
        for mc in range(KC):
            sl = slice(mc * 128, (mc + 1) * 128)
            ps = psum.tile([128, nt], F32, tag="mmps", bufs=4)
            for p2 in range(KC // 2):
                nc.tensor.matmul(ps[:], qstate["W_f18"][:, 2 * p2:2 * p2 + 2, sl],
                                 z_f8[:, 2 * p2:2 * p2 + 2, 0:nt],
                                 start=(p2 == 0), stop=False, perf_mode=DR)
            nc.tensor.matmul(ps[:], qw_tiles[ti][:, sl], ind[:], start=False, stop=True)
            nc.scalar.activation(o_bf[:, mc, :], ps[:], AF.Relu, bias=b_f1c[:, mc:mc + 1],
                                 scale=float(1.0 / FP8_SCALE))
            if store_split:
                # issued right after the ACT so the tail store overlaps f1
                nc.sync.dma_start(outT_r[:, mc, t0:t0 + nt], o_bf[:, mc, :])
        if not store_split:
            nc.sync.dma_start(outT_r[:, :, t0:t0 + nt], o_bf[:])

    qstate = {}

    def emit_Qa():
        qp = ctx.enter_context(tc.tile_pool(name="qpool", bufs=1))
        qstate["qp"] = qp
        if True:
            # sync queue: q-path weights (after x0 so tile0 starts early)
            W_ms1 = qp.tile([128, KC, H], BF16)
            nc.sync.dma_start(W_ms1[:], wi["ms_W1"].rearrange("(kc p) m -> p kc m", p=128))
            W_ms2 = qp.tile([128, KH, D], BF16)
            nc.sync.dma_start(W_ms2[:], wi["ms_W2"].rearrange("(kc p) m -> p kc m", p=128))
            W_seg_a = qp.tile([128, D], BF16)
            nc.sync.dma_start(W_seg_a[:], wi["seg_W"][0:128, :])
            W_seg_b = qp.tile([5, D], BF16)
            nc.sync.dma_start(W_seg_b[:], wi["seg_W"][128:SEG_C, :])
            b_segr = qp.tile([1, D], BF16)
            nc.gpsimd.dma_start(b_segr[:], wi["seg_b"])
            b_ms1r = qp.tile([1, H], BF16)
            nc.gpsimd.dma_start(b_ms1r[:], wi["ms_b1"])
            b_ms2r = qp.tile([1, D], BF16)
            nc.gpsimd.dma_start(b_ms2r[:], wi["ms_b2"])
            g_bf = qp.tile([1, D], BF16)
            nc.gpsimd.dma_start(g_bf[:], wi["ln_g"])
            b_bf = qp.tile([1, D], BF16)
            nc.gpsimd.dma_start(b_bf[:], wi["ln_b"])

            # avgpool(7x7): load batch-major in chunks, reduce, PE-transpose
            pooled = qp.tile([BC, SEG_C], F32)
            CH = 14
            offc = 0
            while offc < SEG_C:
                csz = min(CH, SEG_C - offc)
                ch = qp.tile([BC, CH, 49], BF16, tag="unet_ch", bufs=2)
                nc.gpsimd.dma_start(ch[:, 0:csz, :], unet[:, offc:offc + csz, :])
                nc.vector.reduce_sum(pooled[:, offc:offc + csz], ch[:, 0:csz, :],
                                     axis=mybir.AxisListType.X)
                offc += csz
            pooled_bf = qp.tile([BC, SEG_C], BF16)
            nc.scalar.mul(pooled_bf[:], pooled[:], 1.0 / 49.0)
            # fp8 weights for the DoubleRow stages: loaded here so they sit
            # behind unet in the gpsimd queue, off the startup critical path
            qstate["W_sc8"] = load_w("sc_W8", KC, D, nc.gpsimd, FP8)
            qstate["W_f18"] = load_w("f1_W8", KC, D, nc.gpsimd, FP8)
            qstate.update(pooled_bf=pooled_bf, W_seg_a=W_seg_a, W_seg_b=W_seg_b,
                          b_segr=b_segr, W_ms1=W_ms1, W_ms2=W_ms2,
                          b_ms1r=b_ms1r, b_ms2r=b_ms2r, g_bf=g_bf, b_bf=b_bf)

    def emit_Qa2():
        qp = qstate["qp"]
        pooled_bf = qstate["pooled_bf"]
        W_seg_a, W_seg_b, b_segr = qstate["W_seg_a"], qstate["W_seg_b"], qstate["b_segr"]
        if True:
            pa_ps = psum.tile([128, BC], BF16, tag="tps", bufs=1)
            nc.tensor.transpose(pa_ps[:], pooled_bf[:, 0:128], ident_bf[0:BC, 0:BC])
            pa_bf = qp.tile([128, BC], BF16)
            nc.scalar.copy(pa_bf[:], pa_ps[:])
            pb_ps = psum.tile([5, BC], BF16, tag="tps", bufs=1)
            nc.tensor.transpose(pb_ps[:], pooled_bf[:, 128:SEG_C], ident_bf[0:BC, 0:BC])
            pb_bf = qp.tile([5, BC], BF16)
            nc.scalar.copy(pb_bf[:], pb_ps[:])

            # q1 = relu(pooled @ seg_W + seg_b)   (token-major: BC x D)
            q1 = qp.tile([BC, D], F32)
            for ng in range(2):
                sl = slice(ng * 512, (ng + 1) * 512)
                ps = psum.tile([BC, 512], F32, tag="mmps", bufs=4)
                nc.tensor.matmul(ps[:], pa_bf[:], W_seg_a[:, sl], start=True, stop=False)
                nc.tensor.matmul(ps[:], pb_bf[:], W_seg_b[:, sl], start=False, stop=False)
                nc.tensor.matmul(ps[:], ones_row[0:1, 0:BC], b_segr[0:1, sl], start=False, stop=True)
                nc.vector.tensor_scalar_max(q1[:, sl], ps[:], 0.0)

            # layernorm over D
            stats = qp.tile([BC, 2, 6], F32)
            for s in range(2):
                nc.vector.bn_stats(stats[:, s, :], q1[:, s * 512:(s + 1) * 512])
            mv = qp.tile([BC, 2], F32)
            nc.vector.bn_aggr(mv[:], stats[:])
            rstd = qp.tile([BC, 1], F32)
            nc.scalar.activation(rstd[:], mv[:, 1:2], AF.Sqrt, bias=eps_t[0:BC, :])
            nc.vector.reciprocal(rstd[:], rstd[:])
            # qn here (not Qb) so the whole LN vector chain runs before
            # A2's vector work and Qb's PE never waits on it
            qn = qp.tile([BC, D], F32)
            nc.vector.tensor_scalar(qn[:], q1[:], mv[:, 0:1], rstd[:],
                                    op0=ALU.subtract, op1=ALU.mult)
            qstate.update(qn=qn)

    def emit_Qb():
        qp = qstate["qp"]
        W_ms1, W_ms2 = qstate["W_ms1"], qstate["W_ms2"]
        b_ms1r, b_ms2r = qstate["b_ms1r"], qstate["b_ms2r"]
        g_bf, b_bf = qstate["g_bf"], qstate["b_bf"]
        qn = qstate["qn"]
        # bf16 f1_W (qw matmul only): loaded here, behind x1/x2 on sync
        W_f1 = load_w("f1_W", KC, D, nc.sync)
        if True:
            # apply ln_g / ln_b via K=1 PE broadcasts (ones x row) in PSUM
            qn_bf = qp.tile([BC, D], BF16)
            for ng in range(2):
                sl = slice(ng * 512, (ng + 1) * 512)
                gps = psum.tile([BC, 512], F32, tag="mmps", bufs=4)
                nc.tensor.matmul(gps[:], ones_row[0:1, 0:BC], g_bf[0:1, sl],
                                 start=True, stop=True)
                nc.vector.tensor_mul(qn[:, sl], qn[:, sl], gps[:])
                bps = psum.tile([BC, 512], F32, tag="mmps", bufs=4)
                nc.tensor.matmul(bps[:], ones_row[0:1, 0:BC], b_bf[0:1, sl],
                                 start=True, stop=True)
                nc.vector.tensor_add(qn_bf[:, sl], qn[:, sl], bps[:])

            # qnT (feature-major) via PE transposes
            qnT_bf = qp.tile([128, KC, BC], BF16)
            for kc in range(KC):
                pt = psum.tile([128, BC], BF16, tag="tps", bufs=1)
                nc.tensor.transpose(pt[:], qn_bf[:, kc * 128:(kc + 1) * 128], ident_bf[0:BC, 0:BC])
                nc.scalar.copy(qnT_bf[:, kc, :], pt[:])

            # q MLP, token-major: qm = relu(qn @ ms_W1 + b1)  (BC x H)
            qm_bf = qp.tile([BC, H], BF16)
            ps = psum.tile([BC, H], F32, tag="mmps", bufs=4)
            for kc in range(KC):
                nc.tensor.matmul(ps[:], qnT_bf[:, kc, :], W_ms1[:, kc, :],
                                 start=(kc == 0), stop=False)
            nc.tensor.matmul(ps[:], ones_row[0:1, 0:BC], b_ms1r[0:1, :],
                             start=False, stop=True)
            nc.scalar.activation(qm_bf[:], ps[:], AF.Relu)
            # qmT (feature-major)
            qmT_bf = qp.tile([128, KH, BC], BF16)
            for kh in range(KH):
                pt = psum.tile([128, BC], BF16, tag="tps", bufs=1)
                nc.tensor.transpose(pt[:], qm_bf[:, kh * 128:(kh + 1) * 128], ident_bf[0:BC, 0:BC])
                nc.scalar.copy(qmT_bf[:, kh, :], pt[:])
            # q2 = qm @ ms_W2 + b2 + qn   (token-major)
            q2_bf = qp.tile([BC, D], BF16)
            for ng in range(2):
                sl = slice(ng * 512, (ng + 1) * 512)
                ps = psum.tile([BC, 512], F32, tag="mmps", bufs=4)
                for kh in range(KH):
                    nc.tensor.matmul(ps[:], qmT_bf[:, kh, :], W_ms2[:, kh, sl],
                                     start=(kh == 0), stop=False)
                nc.tensor.matmul(ps[:], ones_row[0:1, 0:BC], b_ms2r[0:1, sl],
                                 start=False, stop=True)
                nc.vector.tensor_add(q2_bf[:, sl], ps[:], qn[:, sl])

            # qT_bf (feature-major q2) via PE transposes
            for kc in range(KC):
                pt = psum.tile([128, BC], BF16, tag="tps", bufs=1)
                nc.tensor.transpose(pt[:], q2_bf[:, kc * 128:(kc + 1) * 128], ident_bf[0:BC, 0:BC])
                nc.scalar.copy(qT_bf[:, kc, :], pt[:])

            # qw = q2 @ f1_W (token-major, no f1_b)
            for ng in range(2):
                sl = slice(ng * 512, (ng + 1) * 512)
                ps = psum.tile([BC, 512], F32, tag="mmps", bufs=4)
                for kc in range(KC):
                    nc.tensor.matmul(ps[:], qT_bf[:, kc, :], W_f1[:, kc, sl],
                                     start=(kc == 0), stop=(kc == KC - 1))
                # x4096 so the f1 PSUM descale also covers the qw part
                nc.scalar.mul(qw_bf[:, sl], ps[:], FP8_SCALE)

            # fill vector for masked tokens: relu(f1_b), from the
            # already-loaded per-partition bias columns
            fb_col = qp.tile([128, KC], F32)
            nc.vector.tensor_scalar_max(fb_col[:], b_f1c[:], 0.0)
            nc.gpsimd.dma_start(
                bass.AP(tensor=fillv.tensor, offset=fillv.offset,
                        ap=[[1, 128], [128, KC]]),
                fb_col[:])

    # ---- pipelined emission --------------------------------------------
    # A0..A2 run before Qb so Q's vector-heavy LN chain hides under their
    # matmuls; B(t) then interleaves with A(t+3). at-matmuls for tiles
    # emitted before Qb (qT not yet written) move into their B instead.
    emit_A(0, with_at=False)
    # x1/x2 go on the sync queue BEFORE Qa's weight loads: A1/A2 must not
    # stall behind 5MB of q-path weights for their input tiles
    for ti in range(1, min(3, T)):
        emit_x(ti)
    emit_Qa()
    if T > 1:
        emit_A(1, with_at=False)
    emit_Qa2()
    if T > 2:
        emit_A(2, with_at=False)
    emit_Qb()
    nextA = 3
    for ti in range(T):
        if nextA < T:
            emit_x(nextA)          # x prefetch ahead of this B's store
        emit_B(ti, with_at=True)
        if nextA < T:
            emit_A(nextA, with_at=False)
            nextA += 1


def _build(plan):
    nc = bacc.Bacc("TRN2", target_bir_lowering=False, debug=False)
    ctx = ExitStack()
    with tile.TileContext(nc) as tc, ctx:
        _emit(ctx, tc, plan)
    nc.compile()
    return nc


_NC_CACHE = {}


def _get_nc(plan_key, plan):
    if plan_key not in _NC_CACHE:
        _NC_CACHE[plan_key] = _build(plan)
    return _NC_CACHE[plan_key]


def _build_ind_blob(tiles):
    sz = sum(t["nb"] * t["nt"] for t in tiles)
    blob = np.zeros(sz, dtype=NPBF16)
    off = 0
    for t in tiles:
        ind = np.zeros((t["nb"], t["nt"]), dtype=NPBF16)
        for row, lo, hi in t["segs"]:
            ind[row, lo:hi] = 1
        blob[off:off + ind.size] = ind.ravel()
        off += ind.size
    return blob


def _run_cores(ncs, in_maps, trace=False):
    """Dispatch one compiled program per core, concurrently."""
    import jax
    from concourse import bass2jax
    from concourse.bass2jax import _bass_exec_p, install_neuronx_cc_hook

    install_neuronx_cc_hook()
    devices = jax.devices()[:NCORES]

    def make_jit(nc):
        in_names, out_names, out_avals, zero_outs = [], [], [], []
        for alloc in nc.m.functions[0].allocations:
            if not isinstance(alloc, mybir.MemoryLocationSet):
                continue
            name = alloc.memorylocations[0].name
            if alloc.kind == "ExternalInput":
                in_names.append(name)
            elif alloc.kind == "ExternalOutput":
                out_names.append(name)
                shape = tuple(alloc.tensor_shape)
                dtype = mybir.dt.np(alloc.dtype)
                out_avals.append(jax.core.ShapedArray(shape, dtype))
                zero_outs.append(np.zeros(shape, dtype))
        n_params = len(in_names)
        all_names = in_names + out_names

        def _body(*args):
            outs = _bass_exec_p.bind(
                *args,
                out_avals=tuple(out_avals),
                in_names=tuple(all_names),
                out_names=tuple(out_names),
                lowering_input_output_aliases=(),
                sim_require_finite=True,
                sim_require_nnan=True,
                nc=nc,
            )
            return tuple(outs)

        donate = tuple(range(n_params, n_params + len(out_names)))
        return (jax.jit(_body, donate_argnums=donate, keep_unused=True),
                in_names, out_names, zero_outs)

    with ThreadPoolExecutor(NCORES) as ex:
        jits = list(ex.map(make_jit, ncs))

    def launch(c):
        jitted, in_names, out_names, zero_outs = jits[c]
        vals = dict(in_maps[c])
        pid = ncs[c].partition_id_tensor
        if pid is not None:
            vals[pid.name] = np.array([[c]], dtype=np.uint32)
        args = [jax.device_put(np.asarray(vals[n]), devices[c]) for n in in_names]
        zz = [jax.device_put(z, devices[c]) for z in zero_outs]
        outs = jitted(*args, *zz)
        return dict(zip(out_names, outs))

    def run_all():
        with ThreadPoolExecutor(NCORES) as ex:
            outs = list(ex.map(launch, range(NCORES)))
        return [{k: np.asarray(v) for k, v in o.items()} for o in outs]

    global LAST_EXEC_NS, _LAST_TRACE
    if trace:
        import glob as globmod
        import tempfile
        from antenv.axon_hooks import get_axon_ntff_profile_hook
        hook = get_axon_ntff_profile_hook()
        neff_dir = tempfile.mkdtemp()
        if hook is None:
            results = run_all()
        else:
            run_all()  # warm: jit trace + NEFF compile before the profiled run
            with hook(neff_dir, [0]):
                results = run_all()
            try:
                import re
                import shutil
                import gauge.profiler
                from concourse._compat import FishPath
                ntffs = sorted(globmod.glob(os.path.join(neff_dir, "*_body*.ntff")))
                times = []
                insts_best = None
                for ntff in ntffs:
                    m = re.search(r"executable(\d+)", os.path.basename(ntff))
                    exe = m.group(1)
                    sub = os.path.join(neff_dir, f"exe{exe}")
                    os.makedirs(sub, exist_ok=True)
                    for fpath in globmod.glob(os.path.join(neff_dir, f"*executable{exe}*")):
                        if os.path.isfile(fpath):
                            shutil.copy(fpath, sub)
                    profile = gauge.profiler.Profile(
                        profile_path=FishPath(sub), kernel_dev_mode=True,
                        profile_on_exit=False, bass_kernel=ncs[0].m,
                        offline_processing=True, fname="*_body*",
                        metadata={"artifacts_path": sub})
                    pr = profile.to_perfetto(model_index=(0,))
                    if pr:
                        times.append(pr[0].exec_time_ns)
                        if pr[0].exec_time_ns == max(times):
                            insts_best = (pr[0].insts, pr[0].trace_path)
                if times:
                    LAST_EXEC_NS = max(times)
                    _LAST_TRACE = insts_best
                    print(f"per-core exec ns: {sorted(times)}", file=sys.stderr)
            except Exception as e:
                print(f"profile post-processing failed: {e!r}", file=sys.stderr)
    else:
        results = run_all()
    return results


def kernel(rgns, Unet_segs, region_lens, mi_W1, mi_b1, mi_W2, mi_b2,
           ms_W1, ms_b1, ms_W2, ms_b2, seg_W, seg_b, ln_g, ln_b,
           sc_W, sc_b, f1_W, f1_b):
    _wire_ntff_hook()

    f = lambda a: np.ascontiguousarray(np.asarray(a, dtype=np.float32))
    bf = lambda a: np.ascontiguousarray(np.asarray(a, dtype=np.float32).astype(NPBF16))
    f8 = lambda a: np.ascontiguousarray(
        np.clip(np.asarray(a, dtype=np.float32) * FP8_SCALE, -240, 240).astype(NPFP8))
    rgns = f(rgns)
    unet = f(Unet_segs).reshape(B, SEG_C, 49)
    lens = np.clip(np.asarray(region_lens).astype(np.int64), 0, R)

    weights = {
        "mi_W1": bf(mi_W1), "mi_b1": f(mi_b1).reshape(1, H),
        "mi_W2": bf(mi_W2), "mi_b2": f(mi_b2).reshape(1, D),
        "ms_W1": bf(ms_W1), "ms_b1": f(ms_b1).reshape(1, H),
        "ms_W2": bf(ms_W2), "ms_b2": f(ms_b2).reshape(1, D),
        "seg_W": bf(seg_W), "seg_b": f(seg_b).reshape(1, D),
        "ln_g": bf(ln_g).reshape(1, D), "ln_b": bf(ln_b).reshape(1, D),
        "sc_W8": f8(sc_W), "sc_b": f(sc_b).reshape(1, D),
        "f1_W": bf(f1_W), "f1_W8": f8(f1_W), "f1_b": f(f1_b).reshape(1, D),
    }

    # balanced batch assignment: 128 batches per core, equalize token counts
    order = np.argsort(-lens, kind="stable")
    loads = np.zeros(NCORES, dtype=np.int64)
    counts = np.zeros(NCORES, dtype=np.int64)
    assign = [[] for _ in range(NCORES)]
    for b in order:
        open_cores = [c for c in range(NCORES) if counts[c] < BC]
        c = min(open_cores, key=lambda c: loads[c])
        assign[c].append(int(b))
        loads[c] += int(lens[b])
        counts[c] += 1
    batches = [np.sort(np.array(a, dtype=np.int64)) for a in assign]

    rflat = rgns.reshape(B * R, D)
    in_maps, plans, vrows = [], [], []
    for c in range(NCORES):
        bl = batches[c]
        lens_c = lens[bl]
        plan = _make_plan(lens_c)
        plans.append(plan)
        rows = np.concatenate([bl[i] * R + np.arange(lens_c[i]) for i in range(BC)])
        vrows.append(rows)
        xTc = np.ascontiguousarray(rflat[rows].astype(NPBF16).T)
        in_maps.append(dict(
            xT=xTc,
            unet=np.ascontiguousarray(unet[bl].astype(NPBF16)),
            ind=_build_ind_blob(plan[0]),
            **weights,
        ))

    def plan_key(c):
        return tuple((t["t0"], t["nt"], t["b0"], t["nb"], tuple(t["segs"]))
                     for t in plans[c][0])

    keys = [plan_key(c) for c in range(NCORES)]
    uniq = {}
    for c in range(NCORES):
        if keys[c] not in uniq:
            uniq[keys[c]] = None
    with ThreadPoolExecutor(min(8, len(uniq))) as ex:
        built = dict(zip(uniq.keys(),
                         ex.map(lambda k: _get_nc(k, plans[keys.index(k)]),
                                list(uniq.keys()))))
    ncs = [built[keys[c]] for c in range(NCORES)]

    trace = bool(int(os.environ.get("BASSK_TRACE", "0")))
    results = _run_cores(ncs, in_maps, trace=trace)

    out = np.empty((B * R, D), np.float32)
    out[:] = results[0]["fillv"].reshape(1, D)
    for c in range(NCORES):
        out[vrows[c]] = results[c]["outT"].T.astype(np.float32)
    return out.reshape(B, R, D)
